# revision 37
# baseline (speedup 1.0000x reference)
"""Trainium2 Bass kernel for a 6-layer GAT GNN (nn_GAT_GNN_35579509080109).

Strategy (8 NeuronCores, node partition):
  - Nodes are degree-balanced into 160 blocks of 128 slots (125 real nodes
    each); each device owns 20 blocks (2560 padded node slots).
  - Per layer, each device computes hw' = h @ (W_l @ B'_l) for its own nodes,
    where B'_l is a Householder rotation whose first column is a_src_l: so
    hw'[:, 0] IS e_src and the table row is just 256 bf16 = 512B (no separate
    e_src, no ones column). B'^{-1} folds into layer l+1's weights host-side
    (layer 5's inverse is applied on-device via binv5 in the final matmuls).
    The table is AllGathered (10.5MB out vs 15.7MB for the old 768B rows).
  - Edges are partitioned by destination owner, sorted into dst blocks, and
    processed in chunks of 128 edges: hw'[src] rows via dma_gather, issued as
    4x256-idx calls round-robined over 4 SWDGE queues (A/B-measured optimum;
    1x1024 on one queue is ~0.7ms/call slower end-to-end - SWDGE descriptor
    generation + completion latency is the dominant real-HW cost here).
    Per-edge index streams (srcw, slotf) are layer-invariant, SBUF-resident.
  - e_dst per edge comes from a host-precomputed static one-hot transpose
    ptT[slot, edge] (fp8, SBUF-resident, 5.2MB): eb = ptT^T @ e_dst_col per
    chunk on the tensor engine. Overlaps the AllGather issue.
  - ee = exp(leaky_relu(e_src+e_dst)) as max(exp(x), exp(0.2x)) on ScalarE.
  - Scatter-add on PE: lt = one-hot(dst slot)*ee (lhsT, bf16) x hw' rows
    (rhs, 256 wide) accumulates [128, 0:256] in PSUM per block; the
    denominator is a second 1-column matmul (same stationary lt, ones rhs)
    into psum[:, 256:257] with start=False ALWAYS - a start=True there would
    re-clear the whole PSUM bank's has_written bits and drop chunk 0's
    messages (hard-won lesson; rel err 0.34 until fixed).
  - Final: h6 = n'5 @ B5^{-1} + b5 on-device (relu can't fold through B), then
    out = relu(h6) @ (W3_top + W3_bot); output DMA batched 4 blocks/call.

Timing notes (this session; measured with drift-cancelling interleaved A/B
on real HW via axon PJRT, wall minus trivial-NEFF floor):
  - Baseline (768B rows, 1 SWDGE queue, gp bufs=5): ~2.45ms delta.
  - 512B rotated rows: -0.65ms (AllGather bytes -33%, gather bytes -33%).
  - 4 SWDGE queues + gp bufs=8: -0.30ms. gsplit 1024->2x512: -0.42ms;
    ->4x256: -0.15ms more; 8x128 regresses +0.26ms. gpbufs 11 ~= 8.
  - Net ~0.9-1.0ms delta vs trivial floor (~2.6x faster than baseline).
  - Tiny-AllGather probe: AG byte cost was ~113us/layer at 768B rows
    (~139GB/s effective); sim's 262us/layer collective model is ~2x high.
  - TimelineSim (trace=True + LazyPerfetto shims, see tlsim.py) showed zero
    compute overlapped the AllGather; the non-AG phase is where real HW ran
    ~2x over the cost model until the SWDGE parallelism fixes.

Closed this session (A/B-measured null or negative):
  - agk=2 two-phase AllGather (split by block half, edges re-partitioned by
    src half, phase-major scatter with bf16 partial flush): +57us. The AG and
    the per-edge gather share HBM bandwidth, so overlapping them creates no
    new bandwidth; the +15% chunk padding and the doubled collective control
    latency eat the rest. The machinery stays behind the "agk" knob (emu.py
    validates it at rel err 1.11e-3) in case the topology changes.
  - The design sits near its HBM roofline: per layer ~34MB of HBM traffic
    (10.5MB AG write + 21MB gather read + table write) ~= 95us/layer floor
    vs ~150us/layer achieved.

Older hard-won constraints that still hold:
  - Do NOT exceed 1024 indices per dma_gather (2048 hangs the device).
  - fp8 table payload lands at rel err 1.7e-2 vs the 2e-2 gate - too close.
  - remote_dma_broadcast receiver-side waits deadlock schedule_block.
  - Strided/sliced collective APs are a NEFF compile reject.
"""
import os
import sys
import numpy as np

for _p in ("/opt/trn_rl_repo", "/root/.axon_site/_ro/trn_rl_repo"):
    if os.path.isdir(_p) and _p not in sys.path:
        sys.path.append(_p)

# ---------------- problem constants ----------------
N = 20000
E = 320000
D = 256
NEG = 0.2
NDEV = 8

GC = 8    # chunks per gather group (1024 edges / dma_gather call; HW limit ~1024 idxs)
RW = 256  # table row width in bf16 (512 bytes): rotated hw' only
# timing-probe knobs (correctness only guaranteed for defaults)
AG_MODE = os.environ.get("KAG", "full")
SKIP = os.environ.get("KSKIP", "")


class Cfg:
    def __init__(self, n, e, bpd):
        self.n, self.e, self.bpd = n, e, bpd
        self.npd = bpd * 128
        self.nblk = NDEV * bpd

FULL = Cfg(N, E, 20)


def _wrap16(flat):
    """dma_gather index layout: idx i at [i%16, i//16], replicated to 128 rows."""
    ni = flat.shape[0]
    w = np.ascontiguousarray(flat.reshape(ni // 16, 16).T).astype(np.int16)
    return np.tile(w, (8, 1))


# ---------------- host preprocessing ----------------
def prep(inputs, cfg, knobs=None):
    kn = {"agk": 1}
    if knobs:
        kn.update(knobs)
    agk = kn["agk"]
    x = np.ascontiguousarray(np.asarray(inputs["x"], np.float32))
    ei = np.asarray(inputs["edge_index"]).astype(np.int64)
    W1 = np.asarray(inputs["W1"], np.float32)
    W2 = np.asarray(inputs["W2"], np.float32)
    Ws = np.asarray(inputs["Ws"], np.float32)
    a_src = np.asarray(inputs["a_src"], np.float32)
    a_dst = np.asarray(inputs["a_dst"], np.float32)
    bias = np.asarray(inputs["bias"], np.float32)
    W3 = np.asarray(inputs["W3"], np.float32)
    src, dst = ei[0], ei[1]
    n, bpd, npd, nblk = cfg.n, cfg.bpd, cfg.npd, cfg.nblk

    # --- degree-balanced node -> (dev, blk, slot) assignment (snake) ---
    deg = np.bincount(dst, minlength=n)
    order = np.argsort(-deg, kind="stable")
    r = np.arange(n)
    stripe = r // nblk
    posin = r % nblk
    blk_glob = np.where(stripe % 2 == 0, posin, nblk - 1 - posin)
    slot = stripe
    assert slot.max() < 128
    pos = np.empty(n, np.int64)
    pos[order] = (blk_glob // bpd) * npd + (blk_glob % bpd) * 128 + slot

    # --- edge grouping by dst block (and src half when agk=2) ---
    dstp = pos[dst]
    srcp_all = pos[src]
    bid = dstp // npd * bpd + (dstp % npd) // 128  # global block id
    nh = npd // 2
    if agk == 2:
        # phase = src's half within its owner device (0: blocks 0..bpd/2-1)
        ph_all = ((srcp_all % npd) >= nh).astype(np.int64)
        skey = bid * 2 + ph_all
        scnt = np.bincount(skey, minlength=nblk * 2)
        cpbA = int(np.ceil(scnt[0::2].max() / 128))
        cpbB = int(np.ceil(scnt[1::2].max() / 128))
        nchA = ((bpd * cpbA + GC - 1) // GC) * GC
        nchB = ((bpd * cpbB + GC - 1) // GC) * GC
        cpb = (cpbA, cpbB)
        nchunk = (nchA, nchB)
        nchunkT = nchA + nchB
    else:
        skey = bid
        scnt = np.bincount(skey, minlength=nblk)
        cpb = int(np.ceil(scnt.max() / 128))
        nchunk = ((bpd * cpb + GC - 1) // GC) * GC
        nchunkT = nchunk
    sidx = np.argsort(skey, kind="stable")
    starts = np.zeros(len(scnt) + 1, np.int64)
    starts[1:] = np.cumsum(scnt)
    rank = np.arange(cfg.e) - starts[skey[sidx]]

    sdev = (dstp // npd)[sidx]
    sblk = ((dstp % npd) // 128)[sidx]
    sslot = (dstp % 128)[sidx]
    if agk == 2:
        sphase = ph_all[sidx]
        srcp_s = srcp_all[sidx]
        # per-phase table row: dev*(npd/2) + offset within half
        ssrc = (srcp_s // npd) * nh + (srcp_s % npd) - sphase * nh
        kk = np.where(
            sphase == 0,
            sblk * cpbA + rank // 128,
            nchA + sblk * cpbB + rank // 128,
        )
    else:
        ssrc = srcp_all[sidx]
        kk = sblk * cpb + rank // 128
    pp = rank % 128

    SRC = np.zeros((NDEV, 128, nchunkT), np.int32)      # per-phase table row
    SLOT = np.full((NDEV, 128, nchunkT), 255.0, np.float32)
    SRC[sdev, pp, kk] = ssrc
    SLOT[sdev, pp, kk] = sslot

    # wrapped int16 index arrays for dma_gather, per group of GC chunks
    ng = nchunkT // GC
    wcols = GC * 128 // 16
    srcw = np.zeros((NDEV, 128, wcols * ng), np.int16)
    for dv in range(NDEV):
        for g in range(ng):
            # edge i in group = c*128 + p, c in [0,GC)
            flat_s = SRC[dv][:, g * GC:(g + 1) * GC].T.reshape(-1)  # [GC*128] c-major
            srcw[dv][:, g * wcols:(g + 1) * wcols] = _wrap16(flat_s)

    # --- x permuted / padded / transposed ---
    xp = np.zeros((NDEV, npd, D), np.float32)
    xp[pos // npd, pos % npd] = x
    xpT = np.ascontiguousarray(xp.transpose(0, 2, 1))

    # --- weights: per-layer rotation folds e_src into hw'[:, 0] ---
    # B'_l = H_l @ diag(||a_src_l||, 1, ...) with Householder H_l e1 =
    # a_src_l/||a_src_l||, so (h @ W_l @ B'_l)[:, 0] = h @ W_l @ a_src_l =
    # e_src and the rest is an orthogonal rotation of hw. The table row is
    # then just 256 bf16 (512B): no separate e_src, no ones column. B'^{-1}
    # folds into layer l+1's weights host-side; layer 5's inverse is applied
    # on-device in the final matmul (binv5).
    W12 = np.ascontiguousarray(W1 @ W2)
    wfull = np.zeros((6, 257, 258), np.float32)
    Binv_prev = None
    for l in range(6):
        u = a_src[l].astype(np.float64)
        nu = float(np.linalg.norm(u))
        e1 = np.zeros(256, np.float64)
        e1[0] = 1.0
        v = u / nu - e1
        vv = float(v @ v)
        H = np.eye(256) - (2.0 / vv) * np.outer(v, v) if vv > 1e-12 else np.eye(256)
        Bp = H.copy()
        Bp[:, 0] *= nu
        Binv = H.copy()
        Binv[0, :] /= nu
        wext = np.concatenate(
            [Ws[l].astype(np.float64) @ Bp,
             (Ws[l] @ a_dst[l]).astype(np.float64)[:, None],
             np.zeros((256, 1))], axis=1
        )  # zero pad col: fp32r matmul needs even free width
        # layer 0 consumes x directly: fold the front MLP (W1 @ W2) in
        if l == 0:
            wfull[l, :256] = (W12.astype(np.float64) @ wext).astype(np.float32)
        else:
            wfull[l, :256] = (Binv_prev @ wext).astype(np.float32)
            wfull[l, 256] = (bias[l - 1].astype(np.float64) @ wext).astype(np.float32)
        Binv_prev = Binv
    W3s = np.ascontiguousarray(W3[:256] + W3[256:])
    binv5 = np.ascontiguousarray(
        Binv_prev.reshape(2, 128, 2, 128).transpose(1, 0, 2, 3)
    ).astype(np.float32)
    b5c = np.ascontiguousarray(bias[5].reshape(2, 128).T).astype(np.float32)
    iotaf = np.tile(np.arange(128, dtype=np.float32)[None, :], (128, 1))
    # static one-hot transpose per chunk: ptT[slot, k*128+e] = (slot(e,k) == slot)
    from concourse import mybir as _mb
    f8 = _mb.dt.np(_mb.dt.float8e4)
    PT8 = np.zeros((NDEV, 128, nchunkT * 128), f8)
    for dv in range(NDEV):
        S = SLOT[dv].astype(np.int32)          # [128 e, nchunk k]
        e_i, k_i = np.nonzero(S < 128)
        PT8[dv][S[e_i, k_i], k_i * 128 + e_i] = 1.0

    in_maps = []
    for dv in range(NDEV):
        in_maps.append(
            {
                "xT": np.ascontiguousarray(xpT[dv]),
                "srcw": np.ascontiguousarray(srcw[dv]),
                "slotf": np.ascontiguousarray(SLOT[dv]),
                "wfull": wfull,
                "w3s": W3s,
                "binv5": binv5,
                "b5c": b5c,
                "iotaf": iotaf,
                "pt8": PT8[dv],
            }
        )
    return in_maps, pos, cpb, nchunk


# ---------------- bass program ----------------
def build(cfg, cpb, nchunk, knobs=None):
    kn = {"swdge": 4, "gpbufs": 8, "gsplit": 4, "psa": 3, "pshwb": 2,
          "expscale": 0, "fuse": 1}
    if knobs:
        kn.update(knobs)
    import concourse.bass as bass
    import concourse.bacc as bacc
    import concourse.tile as tile
    from concourse import mybir
    from concourse.masks import make_identity

    f32 = mybir.dt.float32
    f32r = mybir.dt.float32r
    bf16 = mybir.dt.bfloat16
    i16 = mybir.dt.int16
    AF = mybir.ActivationFunctionType
    OP = mybir.AluOpType
    npd, bpd = cfg.npd, cfg.bpd
    if isinstance(cpb, tuple):
        agk = 2
        cpbA, cpbB = cpb
        nchA, nchB = nchunk
        nchunkT = nchA + nchB
        ngA = nchA // GC
    else:
        agk = 1
        nchunkT = nchunk
        ngA = None
    ng = nchunkT // GC
    nh = npd // 2
    bh = bpd // 2

    def kmap(k):
        """chunk k -> (phase, block, cc, chunks_per_block, valid)"""
        if agk == 1:
            b, cc = divmod(k, cpb)
            return 0, b, cc, cpb, b < bpd
        if k < nchA:
            b, cc = divmod(k, cpbA)
            return 0, b, cc, cpbA, b < bpd
        b, cc = divmod(k - nchA, cpbB)
        return 1, b, cc, cpbB, b < bpd

    nc = bacc.Bacc(
        "TRN2",
        target_bir_lowering=False,
        debug=False,
        enable_asserts=False,
        num_devices=NDEV,
        num_swdge_queues=kn["swdge"],
    )
    xT = nc.dram_tensor("xT", [256, npd], f32, kind="ExternalInput").ap()
    wcols = GC * 128 // 16
    srcw = nc.dram_tensor("srcw", [128, wcols * ng], i16, kind="ExternalInput").ap()
    slotf = nc.dram_tensor("slotf", [128, nchunkT], f32, kind="ExternalInput").ap()
    pt8 = nc.dram_tensor("pt8", [128, nchunkT * 128], mybir.dt.float8e4,
                         kind="ExternalInput").ap()
    wfull = nc.dram_tensor("wfull", [6, 257, 258], f32, kind="ExternalInput").ap()
    w3s = nc.dram_tensor("w3s", [256, 256], f32, kind="ExternalInput").ap()
    binv5 = nc.dram_tensor("binv5", [128, 2, 2, 128], f32, kind="ExternalInput").ap()
    b5c = nc.dram_tensor("b5c", [128, 2], f32, kind="ExternalInput").ap()
    iotaf = nc.dram_tensor("iotaf", [128, 128], f32, kind="ExternalInput").ap()
    out = nc.dram_tensor("out", [npd, 256], f32, kind="ExternalOutput").ap()

    with tile.TileContext(nc) as tc:
        with (
            tc.tile_pool(name="cp", bufs=1) as cp,
            tc.tile_pool(name="sb", bufs=2) as sb,
            tc.tile_pool(name="gp", bufs=kn["gpbufs"]) as gp,
            tc.tile_pool(name="psA", bufs=kn["psa"], space="PSUM") as psA,
            tc.tile_pool(name="psB", bufs=2, space="PSUM") as psB,
            tc.tile_pool(name="dp", bufs=1, space="DRAM") as dp,
        ):
            # ---- constants ----
            ident = cp.tile([128, 128], f32)
            make_identity(nc, ident[:])
            iota_f = cp.tile([128, 128], f32)
            nc.sync.dma_start(iota_f[:], iotaf)
            iota_b = cp.tile([128, 128], bf16)
            nc.vector.tensor_copy(iota_b[:], iota_f[:])
            # layer-invariant edge indexing data, resident in SBUF
            pt8_sb = cp.tile([128, nchunkT * 128], mybir.dt.float8e4)
            nc.sync.dma_start(pt8_sb[:], pt8)
            srcw_all = cp.tile([128, wcols * ng], i16)
            nc.sync.dma_start(srcw_all[:], srcw)
            slot_all = cp.tile([128, nchunkT], f32)
            nc.sync.dma_start(slot_all[:], slotf)
            ones1f = cp.tile([1, 128], f32)
            nc.gpsimd.memset(ones1f[:], 1.0)
            ones1 = cp.tile([1, 128], f32r)
            nc.vector.tensor_copy(ones1[:], ones1f[:])
            onecol = cp.tile([128, 1], bf16)
            nc.gpsimd.memset(onecol[:], 1.0)
            b5sb = cp.tile([128, 2], f32)
            nc.sync.dma_start(b5sb[:], b5c)
            # stage fp32 loads through a small buffer, DVE-cast to f32r
            wextsb = cp.tile([128, 6, 2, 258], f32r)
            bwsb = cp.tile([1, 6, 258], f32r)
            w3ssb = cp.tile([128, 2, 256], f32r)
            binv5sb = cp.tile([128, 2, 2, 128], f32r)

            def _stage(dst_ap, src_ap, shape):
                st = sb.tile(list(shape), f32, tag="wstage", bufs=2, name="wstage")
                nc.sync.dma_start(st[:], src_ap)
                nc.vector.tensor_copy(dst_ap, st[:])

            for l in range(6):
                _stage(
                    wextsb[:, l],
                    wfull[l, 0:256, :].rearrange("(a p) c -> p a c", p=128),
                    [128, 2, 258],
                )
            _stage(bwsb[:], wfull[:, 256:257, :].rearrange("l o c -> o l c"),
                   [1, 6, 258])
            _stage(w3ssb[:], w3s.rearrange("(a p) m -> p a m", p=128), [128, 2, 256])
            _stage(binv5sb[:], binv5, [128, 2, 2, 128])

            # ---- DRAM comm buffers (per layer: Shared tensors allow one writer)
            # agk=2: the table is AllGathered in two block-half pieces so the
            # second collective overlaps phase-A gather/scatter.
            nag = agk
            hrows = npd // nag
            tbl_owns = [
                [dp.tile([hrows, RW], bf16, name=f"tbl_own{i}_{s}")
                 for s in range(nag)]
                for i in range(6)
            ]
            tbl_fulls = [
                [dp.tile([NDEV * hrows, RW], bf16, addr_space="Shared",
                         name=f"tbl_full{i}_{s}")
                 for s in range(nag)]
                for i in range(6)
            ]

            # ---- layer-0 hT is just xT (W1@W2 folded into wfull[0]) ----
            hT = sb.tile([128, 2, npd], f32r, tag="hT")
            xr = xT.rearrange("(a p) n -> p a n", p=128)
            xstep = min(512, npd)
            for n0 in range(0, npd, xstep):
                _stage(hT[:, :, n0:n0 + xstep], xr[:, :, n0:n0 + xstep],
                       [128, 2, xstep])

            def emit_build(l, b, hsrc, tblb, edstb, own, full):
                """hw'(l) for own block b; ship + AllGather after each piece."""
                pshw = psB.tile([128, 258], f32, tag="pshw", bufs=kn["pshwb"])
                for ki in range(2):
                    nc.tensor.matmul(
                        pshw[:],
                        lhsT=hsrc[:, ki, b * 128 : (b + 1) * 128],
                        rhs=wextsb[:, l, ki, :],
                        start=(ki == 0),
                        stop=False,
                    )
                nc.tensor.matmul(
                    pshw[:],
                    lhsT=ones1[:],
                    rhs=bwsb[:, l, :],
                    start=False,
                    stop=True,
                )
                nc.vector.tensor_copy(tblb[:, b, 0:256], pshw[:, 0:256])
                nc.vector.tensor_copy(edstb[:, b : b + 1], pshw[:, 256:257])
                bph = bpd // nag  # blocks per AG piece
                if (b + 1) % bph == 0:
                    s = b // bph
                    nc.sync.dma_start(
                        own[s][:].rearrange("(b p) c -> p b c", p=128),
                        tblb[:, s * bph:(s + 1) * bph],
                    )
                    if AG_MODE == "tiny":
                        nc.gpsimd.collective_compute(
                            "AllGather",
                            mybir.AluOpType.bypass,
                            replica_groups=[list(range(NDEV))],
                            ins=[own[s][0:16]],
                            outs=[full[s][0:128]],
                        )
                    else:
                        nc.gpsimd.collective_compute(
                            "AllGather",
                            mybir.AluOpType.bypass,
                            replica_groups=[list(range(NDEV))],
                            ins=[own[s][:]],
                            outs=[full[s][:]],
                        )

            # ---- layer-0 table: standalone build (nothing to fuse into) ----
            tbl_sb = sb.tile([128, bpd, RW], bf16, tag="tbl")
            edst_all = sb.tile([128, bpd], bf16, tag="edst")
            for b in range(bpd):
                emit_build(0, b, hT, tbl_sb, edst_all, tbl_owns[0], tbl_fulls[0])

            for l in range(6):
                tbl_full = tbl_fulls[l]

                # ---- e_dst per edge for ALL groups (overlaps the AllGather:
                # depends only on edst_all + the static one-hot transpose) ----
                EB_all = sb.tile([128, nchunkT], f32, tag="eball")
                for g in range(ng):
                    pseb = psB.tile([128, GC], f32, tag="pseb", bufs=2)
                    for c in range(GC):
                        k = g * GC + c
                        b = min(kmap(k)[1], bpd - 1)  # clamp for padded chunks
                        nc.tensor.matmul(
                            pseb[:, c : c + 1],
                            lhsT=pt8_sb[:, k * 128:(k + 1) * 128],
                            rhs=edst_all[:, b : b + 1],
                            start=True,
                            stop=True,
                        )
                    nc.vector.tensor_copy(EB_all[:, g * GC:(g + 1) * GC], pseb[:])

                # ---- gather / scatter ----
                hT_next = sb.tile([128, 2, npd], f32r, tag="hT")
                if agk == 2:
                    part = sb.tile([128, bpd, 257], bf16, tag="part", bufs=1)
                if l < 5:
                    tbl_nx = sb.tile([128, bpd, RW], bf16, tag="tbl")
                    edst_nx = sb.tile([128, bpd], bf16, tag="edst")
                pss = None
                for g in range(ng):
                    g_src = tbl_full[0] if (agk == 1 or g < ngA) else tbl_full[1]
                    # --- group prologue: src gather + e_dst broadcast + ee ---
                    G = gp.tile([128, GC, RW], bf16, tag="G")
                    if SKIP != "gather":
                        gs = kn.get("gsplit", 1)
                        cs = GC // gs          # chunks per split
                        ws = cs * 128 // 16    # wrapped idx cols per split
                        for s in range(gs):
                            nc.gpsimd.dma_gather(
                                out_ap=G[:, s * cs:(s + 1) * cs, :],
                                in_ap=g_src[:],
                                idxs_ap=srcw_all[:, g * wcols + s * ws:
                                                  g * wcols + (s + 1) * ws],
                                num_idxs=cs * 128, num_idxs_reg=cs * 128,
                                elem_size=RW,
                                queue_num=(g * gs + s) % kn["swdge"],
                            )
                    if kn["expscale"]:
                        X = gp.tile([128, GC], f32, tag="X")
                        nc.vector.tensor_tensor(
                            X[:], G[:, :, 0],
                            EB_all[:, g * GC:(g + 1) * GC], op=OP.add
                        )
                        E1 = gp.tile([128, 2 * GC], f32, tag="E1")
                        nc.scalar.activation(E1[:, 0:GC], X[:], AF.Exp)
                        nc.scalar.activation(E1[:, GC:2 * GC], X[:], AF.Exp,
                                             scale=NEG)
                        EE = gp.tile([128, GC], f32, tag="EE")
                        nc.vector.tensor_tensor(
                            EE[:], E1[:, 0:GC], E1[:, GC:2 * GC], op=OP.max
                        )
                    else:
                        X = gp.tile([128, 2 * GC], f32, tag="X")
                        nc.vector.tensor_tensor(
                            X[:, 0:GC], G[:, :, 0],
                            EB_all[:, g * GC:(g + 1) * GC], op=OP.add
                        )
                        nc.vector.tensor_scalar(
                            out=X[:, GC:2 * GC], in0=X[:, 0:GC], scalar1=NEG,
                            scalar2=None, op0=OP.mult,
                        )
                        E1 = gp.tile([128, 2 * GC], f32, tag="E1")
                        nc.scalar.activation(E1[:], X[:], AF.Exp)
                        EE = gp.tile([128, GC], f32, tag="EE")
                        nc.vector.tensor_tensor(
                            EE[:], E1[:, 0:GC], E1[:, GC:2 * GC], op=OP.max
                        )
                    # --- scatter pass ---
                    for c in range(GC):
                        k = g * GC + c
                        ph, b, cc, npc, valid = kmap(k)
                        if not valid:
                            continue  # padding chunk (slot=255 rows only)
                        if cc == 0:
                            pss = psA.tile([128, 257], f32, tag="pss")
                        lt = gp.tile([128, 128], bf16, tag="lt")
                        nc.vector.tensor_scalar(
                            out=lt[:],
                            in0=iota_b[:],
                            scalar1=slot_all[:, k : k + 1],
                            scalar2=EE[:, c : c + 1],
                            op0=OP.is_equal,
                            op1=OP.mult,
                        )
                        nc.tensor.matmul(
                            pss[:, 0:256],
                            lhsT=lt[:],
                            rhs=G[:, c, :],
                            start=(cc == 0),
                            stop=(cc == npc - 1),
                        )
                        # denominator column: same stationary lt, ones rhs.
                        # start=False always: the message matmul's start=True
                        # already cleared the whole bank's has_written bits
                        # (a second start here would re-clear them and drop
                        # chunk 0's messages); col 256's bit is clear, so the
                        # first write overwrites, later ones accumulate.
                        nc.tensor.matmul(
                            pss[:, 256:257],
                            lhsT=lt[:],
                            rhs=onecol[:],
                            start=False,
                            stop=(cc == npc - 1),
                        )
                        if cc != npc - 1:
                            continue
                        if agk == 2 and ph == 0:
                            # phase A done for this block: stash partial sums
                            nc.vector.tensor_copy(part[:, b, :], pss[:])
                            continue
                        # ---- block epilogue: normalize ----
                        if agk == 2:
                            tsum = sb.tile([128, 257], f32, tag="tsum")
                            nc.vector.tensor_tensor(
                                tsum[:], pss[:], part[:, b, :], op=OP.add
                            )
                            esrc = tsum
                        else:
                            esrc = pss
                        den = sb.tile([128, 1], f32, tag="den")
                        nc.vector.tensor_scalar(
                            out=den[:], in0=esrc[:, 256:257], scalar1=1e-30,
                            scalar2=None, op0=OP.add,
                        )
                        rec = sb.tile([128, 1], f32, tag="rec")
                        nc.vector.reciprocal(rec[:], den[:])
                        onrm = sb.tile([128, 256], f32, tag="onrm")
                        nc.scalar.activation(
                            onrm[:], esrc[:, 0:256], AF.Copy, scale=rec[:]
                        )
                        srct_t = onrm
                        pst = psB.tile([128, 256], f32, tag="pst", bufs=1)
                        for hh in range(2):
                            nc.tensor.transpose(
                                out=pst[:, hh * 128 : (hh + 1) * 128],
                                in_=srct_t[:, hh * 128 : (hh + 1) * 128],
                                identity=ident[:],
                            )
                            nc.vector.tensor_copy(
                                hT_next[:, hh, b * 128 : (b + 1) * 128],
                                pst[:, hh * 128 : (hh + 1) * 128],
                            )
                        if kn["fuse"] and l < 5:
                            # next layer's table build rides the scatter
                            # stream so its AllGather issues right after the
                            # last epilogue instead of after a build phase
                            emit_build(l + 1, b, hT_next, tbl_nx, edst_nx,
                                       tbl_owns[l + 1], tbl_fulls[l + 1])
                if l < 5 and not kn["fuse"]:
                    for b in range(bpd):
                        emit_build(l + 1, b, hT_next, tbl_nx, edst_nx,
                                   tbl_owns[l + 1], tbl_fulls[l + 1])
                if l < 5:
                    tbl_sb, edst_all = tbl_nx, edst_nx
                hT = hT_next

            # ---- final: h6 = n'5 @ B5^-1 + b5; out = relu(h6) @ (W3_top+W3_bot)
            # batch 4 node blocks per output DMA: fewer HWDGE dispatches
            OB = 4
            for b0 in range(0, bpd, OB):
                nb = min(OB, bpd - b0)
                oo = sb.tile([128, OB, 256], f32, tag="oo")
                for bi in range(nb):
                    b = b0 + bi
                    psf2 = psB.tile([128, 256], f32, tag="pst", bufs=1)
                    for dj in range(2):
                        for ki in range(2):
                            nc.tensor.matmul(
                                psf2[:, dj * 128 : (dj + 1) * 128],
                                lhsT=binv5sb[:, ki, dj],
                                rhs=hT[:, ki, b * 128 : (b + 1) * 128],
                                start=(ki == 0),
                                stop=(ki == 1),
                            )
                    zT = sb.tile([128, 2, 128], f32r, tag="zT")
                    for hh in range(2):
                        nc.vector.tensor_scalar(
                            out=zT[:, hh],
                            in0=psf2[:, hh * 128 : (hh + 1) * 128],
                            scalar1=b5sb[:, hh : hh + 1],
                            scalar2=0.0,
                            op0=OP.add,
                            op1=OP.max,
                        )
                    psf = psB.tile([128, 256], f32, tag="pshw", bufs=kn["pshwb"])
                    for ki in range(2):
                        nc.tensor.matmul(
                            psf[:],
                            lhsT=zT[:, ki],
                            rhs=w3ssb[:, ki, :],
                            start=(ki == 0),
                            stop=(ki == 1),
                        )
                    nc.vector.tensor_copy(oo[:, bi, :], psf[:])
                nc.sync.dma_start(
                    out[b0 * 128 : (b0 + nb) * 128, :].rearrange(
                        "(b p) c -> p b c", p=128
                    ),
                    oo[:, 0:nb, :],
                )

    nc.compile()
    return nc


# ---------------- entry point ----------------
DEFAULT_KNOBS = {}  # build()/prep() internal defaults apply


def kernel(**inputs):
    cfg = FULL
    in_maps, pos, cpb, nchunk = prep(inputs, cfg, knobs=DEFAULT_KNOBS)
    nc = build(cfg, cpb, nchunk, knobs=DEFAULT_KNOBS)
    from concourse import bass_utils

    res = bass_utils.run_bass_kernel_spmd(nc, in_maps, core_ids=list(range(NDEV)))
    outs = [res.results[dv]["out"] for dv in range(NDEV)]
    full = np.zeros((cfg.n, 256), np.float32)
    full[:] = np.stack(outs).reshape(NDEV * cfg.npd, 256)[pos]
    return full



# revision 40
# speedup vs baseline: 1.1461x; 1.1461x over previous
"""Trainium2 Bass kernel for a 6-layer GAT GNN (nn_GAT_GNN_35579509080109).

Strategy (8 NeuronCores, node partition):
  - Nodes are degree-balanced into 160 blocks of 128 slots (125 real nodes
    each); each device owns 20 blocks (2560 padded node slots).
  - Per layer, each device computes hw' = h @ (W_l @ B'_l) for its own nodes,
    where B'_l is a Householder rotation whose first column is a_src_l: so
    hw'[:, 0] IS e_src and the table row is just 256 bf16 = 512B (no separate
    e_src, no ones column). B'^{-1} folds into layer l+1's weights host-side
    (layer 5's inverse is applied on-device via binv5 in the final matmuls).
    The table is AllGathered (10.5MB out vs 15.7MB for the old 768B rows).
  - Edges are partitioned by destination owner, sorted into dst blocks, and
    processed in chunks of 128 edges: hw'[src] rows via dma_gather, issued as
    4x256-idx calls round-robined over 4 SWDGE queues (A/B-measured optimum;
    1x1024 on one queue is ~0.7ms/call slower end-to-end - SWDGE descriptor
    generation + completion latency is the dominant real-HW cost here).
    Per-edge index streams (srcw, slotf) are layer-invariant, SBUF-resident.
  - e_dst per edge comes from a host-precomputed static one-hot transpose
    ptT[slot, edge] (fp8, SBUF-resident, 5.2MB): eb = ptT^T @ e_dst_col per
    chunk on the tensor engine. Overlaps the AllGather issue.
  - ee = exp(leaky_relu(e_src+e_dst)) as max(exp(x), exp(0.2x)) on ScalarE.
  - Scatter-add on PE: lt = one-hot(dst slot)*ee (lhsT, bf16) x hw' rows
    (rhs, 256 wide) accumulates [128, 0:256] in PSUM per block; the
    denominator is a second 1-column matmul (same stationary lt, ones rhs)
    into psum[:, 256:257] with start=False ALWAYS - a start=True there would
    re-clear the whole PSUM bank's has_written bits and drop chunk 0's
    messages (hard-won lesson; rel err 0.34 until fixed).
  - Final: h6 = n'5 @ B5^{-1} + b5 on-device (relu can't fold through B), then
    out = relu(h6) @ (W3_top + W3_bot); output DMA batched 4 blocks/call.
  - Layer l+1's table build is FUSED into layer l's scatter epilogues
    (emit_build after each block's hT_next write), so the next AllGather
    issues right after the last epilogue instead of after a separate build
    phase (A/B: -53us total).

Timing notes (this session; measured with drift-cancelling interleaved A/B
on real HW via axon PJRT, wall minus trivial-NEFF floor):
  - Baseline (768B rows, 1 SWDGE queue, gp bufs=5): ~2.45ms delta.
  - 512B rotated rows: -0.65ms (AllGather bytes -33%, gather bytes -33%).
  - 4 SWDGE queues + gp bufs=8: -0.30ms. gsplit 1024->2x512: -0.42ms;
    ->4x256: -0.15ms more; 8x128 regresses +0.26ms. gpbufs 11 ~= 8.
  - Net ~0.9-1.0ms delta vs trivial floor (~2.6x faster than baseline).
  - Tiny-AllGather probe: AG byte cost was ~113us/layer at 768B rows
    (~139GB/s effective); sim's 262us/layer collective model is ~2x high.
  - TimelineSim (trace=True + LazyPerfetto shims, see tlsim.py) showed zero
    compute overlapped the AllGather; the non-AG phase is where real HW ran
    ~2x over the cost model until the SWDGE parallelism fixes.

Closed this session (A/B-measured null or negative):
  - agk=2 two-phase AllGather (split by block half, edges re-partitioned by
    src half, phase-major scatter with bf16 partial flush): +57us. Each
    collective carries ~40us of control latency (TOPSP stepping) regardless
    of size, so splitting doubles that and cancels the overlap win; HBM
    bandwidth is also shared between the AG and the gather. The machinery
    stays behind the "agk" knob (emu.py validates it at rel err 1.11e-3).
  - Final-output fusion into layer 5's epilogues ("fusef" knob): -1us (the
    scheduler already hides the tail). GC=16 one-block groups as 8x256-idx
    gathers ("gc" via ab.py): -26us, within noise. Deeper PSUM rotation
    (psa 4) + ACT-side exp-scale: +44us. All dormant behind knobs.
  - The design sits near its serial-dependency floor: per layer ~75us
    AllGather (about half irreducible control latency) + ~60us gather
    (byte-bound: 21MB/layer at ~358GB/s HBM) with build/EB/epilogues hidden
    under them; ~34MB total HBM traffic/layer ~= 95us pure-bandwidth floor.

Older hard-won constraints that still hold:
  - Do NOT exceed 1024 indices per dma_gather (2048 hangs the device).
  - fp8 table payload lands at rel err 1.7e-2 vs the 2e-2 gate - too close.
  - remote_dma_broadcast receiver-side waits deadlock schedule_block.
  - Strided/sliced collective APs are a NEFF compile reject.
"""
import os
import sys
import numpy as np

for _p in ("/opt/trn_rl_repo", "/root/.axon_site/_ro/trn_rl_repo"):
    if os.path.isdir(_p) and _p not in sys.path:
        sys.path.append(_p)

# ---------------- problem constants ----------------
N = 20000
E = 320000
D = 256
NEG = 0.2
NDEV = 8

GC = 8    # chunks per gather group (1024 edges / dma_gather call; HW limit ~1024 idxs)
RW = 256  # table row width in bf16 (512 bytes): rotated hw' only
# timing-probe knobs (correctness only guaranteed for defaults)
AG_MODE = os.environ.get("KAG", "full")
SKIP = os.environ.get("KSKIP", "")


class Cfg:
    def __init__(self, n, e, bpd):
        self.n, self.e, self.bpd = n, e, bpd
        self.npd = bpd * 128
        self.nblk = NDEV * bpd

FULL = Cfg(N, E, 20)


def _wrap16(flat):
    """dma_gather index layout: idx i at [i%16, i//16], replicated to 128 rows."""
    ni = flat.shape[0]
    w = np.ascontiguousarray(flat.reshape(ni // 16, 16).T).astype(np.int16)
    return np.tile(w, (8, 1))


# ---------------- host preprocessing ----------------
def prep(inputs, cfg, knobs=None):
    kn = {"agk": 1}
    if knobs:
        kn.update(knobs)
    agk = kn["agk"]
    x = np.ascontiguousarray(np.asarray(inputs["x"], np.float32))
    ei = np.asarray(inputs["edge_index"]).astype(np.int64)
    W1 = np.asarray(inputs["W1"], np.float32)
    W2 = np.asarray(inputs["W2"], np.float32)
    Ws = np.asarray(inputs["Ws"], np.float32)
    a_src = np.asarray(inputs["a_src"], np.float32)
    a_dst = np.asarray(inputs["a_dst"], np.float32)
    bias = np.asarray(inputs["bias"], np.float32)
    W3 = np.asarray(inputs["W3"], np.float32)
    src, dst = ei[0], ei[1]
    n, bpd, npd, nblk = cfg.n, cfg.bpd, cfg.npd, cfg.nblk

    # --- degree-balanced node -> (dev, blk, slot) assignment (snake) ---
    deg = np.bincount(dst, minlength=n)
    order = np.argsort(-deg, kind="stable")
    r = np.arange(n)
    stripe = r // nblk
    posin = r % nblk
    blk_glob = np.where(stripe % 2 == 0, posin, nblk - 1 - posin)
    slot = stripe
    assert slot.max() < 128
    pos = np.empty(n, np.int64)
    pos[order] = (blk_glob // bpd) * npd + (blk_glob % bpd) * 128 + slot

    # --- edge grouping by dst block (and src half when agk=2) ---
    dstp = pos[dst]
    srcp_all = pos[src]
    bid = dstp // npd * bpd + (dstp % npd) // 128  # global block id
    nh = npd // 2
    if agk == 2:
        # phase = src's half within its owner device (0: blocks 0..bpd/2-1)
        ph_all = ((srcp_all % npd) >= nh).astype(np.int64)
        skey = bid * 2 + ph_all
        scnt = np.bincount(skey, minlength=nblk * 2)
        cpbA = int(np.ceil(scnt[0::2].max() / 128))
        cpbB = int(np.ceil(scnt[1::2].max() / 128))
        nchA = ((bpd * cpbA + GC - 1) // GC) * GC
        nchB = ((bpd * cpbB + GC - 1) // GC) * GC
        cpb = (cpbA, cpbB)
        nchunk = (nchA, nchB)
        nchunkT = nchA + nchB
    else:
        skey = bid
        scnt = np.bincount(skey, minlength=nblk)
        cpb = int(np.ceil(scnt.max() / 128))
        nchunk = ((bpd * cpb + GC - 1) // GC) * GC
        nchunkT = nchunk
    sidx = np.argsort(skey, kind="stable")
    starts = np.zeros(len(scnt) + 1, np.int64)
    starts[1:] = np.cumsum(scnt)
    rank = np.arange(cfg.e) - starts[skey[sidx]]

    sdev = (dstp // npd)[sidx]
    sblk = ((dstp % npd) // 128)[sidx]
    sslot = (dstp % 128)[sidx]
    if agk == 2:
        sphase = ph_all[sidx]
        srcp_s = srcp_all[sidx]
        # per-phase table row: dev*(npd/2) + offset within half
        ssrc = (srcp_s // npd) * nh + (srcp_s % npd) - sphase * nh
        kk = np.where(
            sphase == 0,
            sblk * cpbA + rank // 128,
            nchA + sblk * cpbB + rank // 128,
        )
    else:
        ssrc = srcp_all[sidx]
        kk = sblk * cpb + rank // 128
    pp = rank % 128

    SRC = np.zeros((NDEV, 128, nchunkT), np.int32)      # per-phase table row
    SLOT = np.full((NDEV, 128, nchunkT), 255.0, np.float32)
    SRC[sdev, pp, kk] = ssrc
    SLOT[sdev, pp, kk] = sslot

    # wrapped int16 index arrays for dma_gather, per group of GC chunks
    ng = nchunkT // GC
    wcols = GC * 128 // 16
    srcw = np.zeros((NDEV, 128, wcols * ng), np.int16)
    for dv in range(NDEV):
        for g in range(ng):
            # edge i in group = c*128 + p, c in [0,GC)
            flat_s = SRC[dv][:, g * GC:(g + 1) * GC].T.reshape(-1)  # [GC*128] c-major
            srcw[dv][:, g * wcols:(g + 1) * wcols] = _wrap16(flat_s)

    # --- x permuted / padded / transposed ---
    xp = np.zeros((NDEV, npd, D), np.float32)
    xp[pos // npd, pos % npd] = x
    xpT = np.ascontiguousarray(xp.transpose(0, 2, 1))

    # --- weights: per-layer rotation folds e_src into hw'[:, 0] ---
    # B'_l = H_l @ diag(||a_src_l||, 1, ...) with Householder H_l e1 =
    # a_src_l/||a_src_l||, so (h @ W_l @ B'_l)[:, 0] = h @ W_l @ a_src_l =
    # e_src and the rest is an orthogonal rotation of hw. The table row is
    # then just 256 bf16 (512B): no separate e_src, no ones column. B'^{-1}
    # folds into layer l+1's weights host-side; layer 5's inverse is applied
    # on-device in the final matmul (binv5).
    W12 = np.ascontiguousarray(W1 @ W2)
    wfull = np.zeros((6, 257, 258), np.float32)
    Binv_prev = None
    for l in range(6):
        u = a_src[l].astype(np.float64)
        nu = float(np.linalg.norm(u))
        e1 = np.zeros(256, np.float64)
        e1[0] = 1.0
        v = u / nu - e1
        vv = float(v @ v)
        H = np.eye(256) - (2.0 / vv) * np.outer(v, v) if vv > 1e-12 else np.eye(256)
        Bp = H.copy()
        Bp[:, 0] *= nu
        Binv = H.copy()
        Binv[0, :] /= nu
        wext = np.concatenate(
            [Ws[l].astype(np.float64) @ Bp,
             (Ws[l] @ a_dst[l]).astype(np.float64)[:, None],
             np.zeros((256, 1))], axis=1
        )  # zero pad col: fp32r matmul needs even free width
        # layer 0 consumes x directly: fold the front MLP (W1 @ W2) in
        if l == 0:
            wfull[l, :256] = (W12.astype(np.float64) @ wext).astype(np.float32)
        else:
            wfull[l, :256] = (Binv_prev @ wext).astype(np.float32)
            wfull[l, 256] = (bias[l - 1].astype(np.float64) @ wext).astype(np.float32)
        Binv_prev = Binv
    W3s = np.ascontiguousarray(W3[:256] + W3[256:])
    binv5 = np.ascontiguousarray(
        Binv_prev.reshape(2, 128, 2, 128).transpose(1, 0, 2, 3)
    ).astype(np.float32)
    b5c = np.ascontiguousarray(bias[5].reshape(2, 128).T).astype(np.float32)
    iotaf = np.tile(np.arange(128, dtype=np.float32)[None, :], (128, 1))
    # static one-hot transpose per chunk: ptT[slot, k*128+e] = (slot(e,k) == slot)
    from concourse import mybir as _mb
    f8 = _mb.dt.np(_mb.dt.float8e4)
    PT8 = np.zeros((NDEV, 128, nchunkT * 128), f8)
    for dv in range(NDEV):
        S = SLOT[dv].astype(np.int32)          # [128 e, nchunk k]
        e_i, k_i = np.nonzero(S < 128)
        PT8[dv][S[e_i, k_i], k_i * 128 + e_i] = 1.0

    in_maps = []
    for dv in range(NDEV):
        in_maps.append(
            {
                "xT": np.ascontiguousarray(xpT[dv]),
                "srcw": np.ascontiguousarray(srcw[dv]),
                "slotf": np.ascontiguousarray(SLOT[dv]),
                "wfull": wfull,
                "w3s": W3s,
                "binv5": binv5,
                "b5c": b5c,
                "iotaf": iotaf,
                "pt8": PT8[dv],
            }
        )
    return in_maps, pos, cpb, nchunk


# ---------------- bass program ----------------
def build(cfg, cpb, nchunk, knobs=None):
    kn = {"swdge": 4, "gpbufs": 8, "gsplit": 4, "psa": 3, "pshwb": 2,
          "expscale": 0, "fuse": 1, "fusef": 0}
    if knobs:
        kn.update(knobs)
    import concourse.bass as bass
    import concourse.bacc as bacc
    import concourse.tile as tile
    from concourse import mybir
    from concourse.masks import make_identity

    f32 = mybir.dt.float32
    f32r = mybir.dt.float32r
    bf16 = mybir.dt.bfloat16
    i16 = mybir.dt.int16
    AF = mybir.ActivationFunctionType
    OP = mybir.AluOpType
    npd, bpd = cfg.npd, cfg.bpd
    if isinstance(cpb, tuple):
        agk = 2
        cpbA, cpbB = cpb
        nchA, nchB = nchunk
        nchunkT = nchA + nchB
        ngA = nchA // GC
    else:
        agk = 1
        nchunkT = nchunk
        ngA = None
    ng = nchunkT // GC
    nh = npd // 2
    bh = bpd // 2

    def kmap(k):
        """chunk k -> (phase, block, cc, chunks_per_block, valid)"""
        if agk == 1:
            b, cc = divmod(k, cpb)
            return 0, b, cc, cpb, b < bpd
        if k < nchA:
            b, cc = divmod(k, cpbA)
            return 0, b, cc, cpbA, b < bpd
        b, cc = divmod(k - nchA, cpbB)
        return 1, b, cc, cpbB, b < bpd

    nc = bacc.Bacc(
        "TRN2",
        target_bir_lowering=False,
        debug=False,
        enable_asserts=False,
        num_devices=NDEV,
        num_swdge_queues=kn["swdge"],
    )
    xT = nc.dram_tensor("xT", [256, npd], f32, kind="ExternalInput").ap()
    wcols = GC * 128 // 16
    srcw = nc.dram_tensor("srcw", [128, wcols * ng], i16, kind="ExternalInput").ap()
    slotf = nc.dram_tensor("slotf", [128, nchunkT], f32, kind="ExternalInput").ap()
    pt8 = nc.dram_tensor("pt8", [128, nchunkT * 128], mybir.dt.float8e4,
                         kind="ExternalInput").ap()
    wfull = nc.dram_tensor("wfull", [6, 257, 258], f32, kind="ExternalInput").ap()
    w3s = nc.dram_tensor("w3s", [256, 256], f32, kind="ExternalInput").ap()
    binv5 = nc.dram_tensor("binv5", [128, 2, 2, 128], f32, kind="ExternalInput").ap()
    b5c = nc.dram_tensor("b5c", [128, 2], f32, kind="ExternalInput").ap()
    iotaf = nc.dram_tensor("iotaf", [128, 128], f32, kind="ExternalInput").ap()
    out = nc.dram_tensor("out", [npd, 256], f32, kind="ExternalOutput").ap()

    with tile.TileContext(nc) as tc:
        with (
            tc.tile_pool(name="cp", bufs=1) as cp,
            tc.tile_pool(name="sb", bufs=2) as sb,
            tc.tile_pool(name="gp", bufs=kn["gpbufs"]) as gp,
            tc.tile_pool(name="psA", bufs=kn["psa"], space="PSUM") as psA,
            tc.tile_pool(name="psB", bufs=2, space="PSUM") as psB,
            tc.tile_pool(name="dp", bufs=1, space="DRAM") as dp,
        ):
            # ---- constants ----
            ident = cp.tile([128, 128], f32)
            make_identity(nc, ident[:])
            iota_f = cp.tile([128, 128], f32)
            nc.sync.dma_start(iota_f[:], iotaf)
            iota_b = cp.tile([128, 128], bf16)
            nc.vector.tensor_copy(iota_b[:], iota_f[:])
            # layer-invariant edge indexing data, resident in SBUF
            pt8_sb = cp.tile([128, nchunkT * 128], mybir.dt.float8e4)
            nc.sync.dma_start(pt8_sb[:], pt8)
            srcw_all = cp.tile([128, wcols * ng], i16)
            nc.sync.dma_start(srcw_all[:], srcw)
            slot_all = cp.tile([128, nchunkT], f32)
            nc.sync.dma_start(slot_all[:], slotf)
            ones1f = cp.tile([1, 128], f32)
            nc.gpsimd.memset(ones1f[:], 1.0)
            ones1 = cp.tile([1, 128], f32r)
            nc.vector.tensor_copy(ones1[:], ones1f[:])
            onecol = cp.tile([128, 1], bf16)
            nc.gpsimd.memset(onecol[:], 1.0)
            b5sb = cp.tile([128, 2], f32)
            nc.sync.dma_start(b5sb[:], b5c)
            # stage fp32 loads through a small buffer, DVE-cast to f32r
            wextsb = cp.tile([128, 6, 2, 258], f32r)
            bwsb = cp.tile([1, 6, 258], f32r)
            w3ssb = cp.tile([128, 2, 256], f32r)
            binv5sb = cp.tile([128, 2, 2, 128], f32r)

            def _stage(dst_ap, src_ap, shape):
                st = sb.tile(list(shape), f32, tag="wstage", bufs=2, name="wstage")
                nc.sync.dma_start(st[:], src_ap)
                nc.vector.tensor_copy(dst_ap, st[:])

            for l in range(6):
                _stage(
                    wextsb[:, l],
                    wfull[l, 0:256, :].rearrange("(a p) c -> p a c", p=128),
                    [128, 2, 258],
                )
            _stage(bwsb[:], wfull[:, 256:257, :].rearrange("l o c -> o l c"),
                   [1, 6, 258])
            _stage(w3ssb[:], w3s.rearrange("(a p) m -> p a m", p=128), [128, 2, 256])
            _stage(binv5sb[:], binv5, [128, 2, 2, 128])

            # ---- DRAM comm buffers (per layer: Shared tensors allow one writer)
            # agk=2: the table is AllGathered in two block-half pieces so the
            # second collective overlaps phase-A gather/scatter.
            nag = agk
            hrows = npd // nag
            tbl_owns = [
                [dp.tile([hrows, RW], bf16, name=f"tbl_own{i}_{s}")
                 for s in range(nag)]
                for i in range(6)
            ]
            tbl_fulls = [
                [dp.tile([NDEV * hrows, RW], bf16, addr_space="Shared",
                         name=f"tbl_full{i}_{s}")
                 for s in range(nag)]
                for i in range(6)
            ]

            # ---- layer-0 hT is just xT (W1@W2 folded into wfull[0]) ----
            hT = sb.tile([128, 2, npd], f32r, tag="hT")
            xr = xT.rearrange("(a p) n -> p a n", p=128)
            xstep = min(512, npd)
            for n0 in range(0, npd, xstep):
                _stage(hT[:, :, n0:n0 + xstep], xr[:, :, n0:n0 + xstep],
                       [128, 2, xstep])

            def emit_build(l, b, hsrc, tblb, edstb, own, full):
                """hw'(l) for own block b; ship + AllGather after each piece."""
                pshw = psB.tile([128, 258], f32, tag="pshw", bufs=kn["pshwb"])
                for ki in range(2):
                    nc.tensor.matmul(
                        pshw[:],
                        lhsT=hsrc[:, ki, b * 128 : (b + 1) * 128],
                        rhs=wextsb[:, l, ki, :],
                        start=(ki == 0),
                        stop=False,
                    )
                nc.tensor.matmul(
                    pshw[:],
                    lhsT=ones1[:],
                    rhs=bwsb[:, l, :],
                    start=False,
                    stop=True,
                )
                nc.vector.tensor_copy(tblb[:, b, 0:256], pshw[:, 0:256])
                nc.vector.tensor_copy(edstb[:, b : b + 1], pshw[:, 256:257])
                bph = bpd // nag  # blocks per AG piece
                if (b + 1) % bph == 0:
                    s = b // bph
                    nc.sync.dma_start(
                        own[s][:].rearrange("(b p) c -> p b c", p=128),
                        tblb[:, s * bph:(s + 1) * bph],
                    )
                    if AG_MODE == "tiny":
                        nc.gpsimd.collective_compute(
                            "AllGather",
                            mybir.AluOpType.bypass,
                            replica_groups=[list(range(NDEV))],
                            ins=[own[s][0:16]],
                            outs=[full[s][0:128]],
                        )
                    else:
                        nc.gpsimd.collective_compute(
                            "AllGather",
                            mybir.AluOpType.bypass,
                            replica_groups=[list(range(NDEV))],
                            ins=[own[s][:]],
                            outs=[full[s][:]],
                        )

            OB = 4  # node blocks per output DMA

            def emit_final(b, hsrc, oo_t, bi):
                """h6 = n'5 @ B5^-1 + b5; out = relu(h6) @ (W3_top+W3_bot)."""
                psf2 = psB.tile([128, 256], f32, tag="pst", bufs=1)
                for dj in range(2):
                    for ki in range(2):
                        nc.tensor.matmul(
                            psf2[:, dj * 128 : (dj + 1) * 128],
                            lhsT=binv5sb[:, ki, dj],
                            rhs=hsrc[:, ki, b * 128 : (b + 1) * 128],
                            start=(ki == 0),
                            stop=(ki == 1),
                        )
                zT = sb.tile([128, 2, 128], f32r, tag="zT")
                for hh in range(2):
                    nc.vector.tensor_scalar(
                        out=zT[:, hh],
                        in0=psf2[:, hh * 128 : (hh + 1) * 128],
                        scalar1=b5sb[:, hh : hh + 1],
                        scalar2=0.0,
                        op0=OP.add,
                        op1=OP.max,
                    )
                psf = psB.tile([128, 256], f32, tag="pshw", bufs=kn["pshwb"])
                for ki in range(2):
                    nc.tensor.matmul(
                        psf[:],
                        lhsT=zT[:, ki],
                        rhs=w3ssb[:, ki, :],
                        start=(ki == 0),
                        stop=(ki == 1),
                    )
                nc.vector.tensor_copy(oo_t[:, bi, :], psf[:])
                if bi == OB - 1 or b == bpd - 1:
                    b0 = b - bi
                    nc.sync.dma_start(
                        out[b0 * 128 : (b + 1) * 128, :].rearrange(
                            "(b p) c -> p b c", p=128
                        ),
                        oo_t[:, 0 : bi + 1, :],
                    )

            # ---- layer-0 table: standalone build (nothing to fuse into) ----
            tbl_sb = sb.tile([128, bpd, RW], bf16, tag="tbl")
            edst_all = sb.tile([128, bpd], bf16, tag="edst")
            for b in range(bpd):
                emit_build(0, b, hT, tbl_sb, edst_all, tbl_owns[0], tbl_fulls[0])

            for l in range(6):
                tbl_full = tbl_fulls[l]

                # ---- e_dst per edge for ALL groups (overlaps the AllGather:
                # depends only on edst_all + the static one-hot transpose) ----
                EB_all = sb.tile([128, nchunkT], f32, tag="eball")
                for g in range(ng):
                    pseb = psB.tile([128, GC], f32, tag="pseb", bufs=2)
                    for c in range(GC):
                        k = g * GC + c
                        b = min(kmap(k)[1], bpd - 1)  # clamp for padded chunks
                        nc.tensor.matmul(
                            pseb[:, c : c + 1],
                            lhsT=pt8_sb[:, k * 128:(k + 1) * 128],
                            rhs=edst_all[:, b : b + 1],
                            start=True,
                            stop=True,
                        )
                    nc.vector.tensor_copy(EB_all[:, g * GC:(g + 1) * GC], pseb[:])

                # ---- gather / scatter ----
                hT_next = sb.tile([128, 2, npd], f32r, tag="hT")
                if agk == 2:
                    part = sb.tile([128, bpd, 257], bf16, tag="part", bufs=1)
                if l < 5:
                    tbl_nx = sb.tile([128, bpd, RW], bf16, tag="tbl")
                    edst_nx = sb.tile([128, bpd], bf16, tag="edst")
                pss = None
                for g in range(ng):
                    g_src = tbl_full[0] if (agk == 1 or g < ngA) else tbl_full[1]
                    # --- group prologue: src gather + e_dst broadcast + ee ---
                    G = gp.tile([128, GC, RW], bf16, tag="G")
                    if SKIP != "gather":
                        gs = kn.get("gsplit", 1)
                        cs = GC // gs          # chunks per split
                        ws = cs * 128 // 16    # wrapped idx cols per split
                        for s in range(gs):
                            nc.gpsimd.dma_gather(
                                out_ap=G[:, s * cs:(s + 1) * cs, :],
                                in_ap=g_src[:],
                                idxs_ap=srcw_all[:, g * wcols + s * ws:
                                                  g * wcols + (s + 1) * ws],
                                num_idxs=cs * 128, num_idxs_reg=cs * 128,
                                elem_size=RW,
                                queue_num=(g * gs + s) % kn["swdge"],
                            )
                    if kn["expscale"]:
                        X = gp.tile([128, GC], f32, tag="X")
                        nc.vector.tensor_tensor(
                            X[:], G[:, :, 0],
                            EB_all[:, g * GC:(g + 1) * GC], op=OP.add
                        )
                        E1 = gp.tile([128, 2 * GC], f32, tag="E1")
                        nc.scalar.activation(E1[:, 0:GC], X[:], AF.Exp)
                        nc.scalar.activation(E1[:, GC:2 * GC], X[:], AF.Exp,
                                             scale=NEG)
                        EE = gp.tile([128, GC], f32, tag="EE")
                        nc.vector.tensor_tensor(
                            EE[:], E1[:, 0:GC], E1[:, GC:2 * GC], op=OP.max
                        )
                    else:
                        X = gp.tile([128, 2 * GC], f32, tag="X")
                        nc.vector.tensor_tensor(
                            X[:, 0:GC], G[:, :, 0],
                            EB_all[:, g * GC:(g + 1) * GC], op=OP.add
                        )
                        nc.vector.tensor_scalar(
                            out=X[:, GC:2 * GC], in0=X[:, 0:GC], scalar1=NEG,
                            scalar2=None, op0=OP.mult,
                        )
                        E1 = gp.tile([128, 2 * GC], f32, tag="E1")
                        nc.scalar.activation(E1[:], X[:], AF.Exp)
                        EE = gp.tile([128, GC], f32, tag="EE")
                        nc.vector.tensor_tensor(
                            EE[:], E1[:, 0:GC], E1[:, GC:2 * GC], op=OP.max
                        )
                    # --- scatter pass ---
                    for c in range(GC):
                        k = g * GC + c
                        ph, b, cc, npc, valid = kmap(k)
                        if not valid:
                            continue  # padding chunk (slot=255 rows only)
                        if cc == 0:
                            pss = psA.tile([128, 257], f32, tag="pss")
                        lt = gp.tile([128, 128], bf16, tag="lt")
                        nc.vector.tensor_scalar(
                            out=lt[:],
                            in0=iota_b[:],
                            scalar1=slot_all[:, k : k + 1],
                            scalar2=EE[:, c : c + 1],
                            op0=OP.is_equal,
                            op1=OP.mult,
                        )
                        nc.tensor.matmul(
                            pss[:, 0:256],
                            lhsT=lt[:],
                            rhs=G[:, c, :],
                            start=(cc == 0),
                            stop=(cc == npc - 1),
                        )
                        # denominator column: same stationary lt, ones rhs.
                        # start=False always: the message matmul's start=True
                        # already cleared the whole bank's has_written bits
                        # (a second start here would re-clear them and drop
                        # chunk 0's messages); col 256's bit is clear, so the
                        # first write overwrites, later ones accumulate.
                        nc.tensor.matmul(
                            pss[:, 256:257],
                            lhsT=lt[:],
                            rhs=onecol[:],
                            start=False,
                            stop=(cc == npc - 1),
                        )
                        if cc != npc - 1:
                            continue
                        if agk == 2 and ph == 0:
                            # phase A done for this block: stash partial sums
                            nc.vector.tensor_copy(part[:, b, :], pss[:])
                            continue
                        # ---- block epilogue: normalize ----
                        if agk == 2:
                            tsum = sb.tile([128, 257], f32, tag="tsum")
                            nc.vector.tensor_tensor(
                                tsum[:], pss[:], part[:, b, :], op=OP.add
                            )
                            esrc = tsum
                        else:
                            esrc = pss
                        den = sb.tile([128, 1], f32, tag="den")
                        nc.vector.tensor_scalar(
                            out=den[:], in0=esrc[:, 256:257], scalar1=1e-30,
                            scalar2=None, op0=OP.add,
                        )
                        rec = sb.tile([128, 1], f32, tag="rec")
                        nc.vector.reciprocal(rec[:], den[:])
                        onrm = sb.tile([128, 256], f32, tag="onrm")
                        nc.scalar.activation(
                            onrm[:], esrc[:, 0:256], AF.Copy, scale=rec[:]
                        )
                        srct_t = onrm
                        pst = psB.tile([128, 256], f32, tag="pst", bufs=1)
                        for hh in range(2):
                            nc.tensor.transpose(
                                out=pst[:, hh * 128 : (hh + 1) * 128],
                                in_=srct_t[:, hh * 128 : (hh + 1) * 128],
                                identity=ident[:],
                            )
                            nc.vector.tensor_copy(
                                hT_next[:, hh, b * 128 : (b + 1) * 128],
                                pst[:, hh * 128 : (hh + 1) * 128],
                            )
                        if kn["fuse"] and l < 5:
                            # next layer's table build rides the scatter
                            # stream so its AllGather issues right after the
                            # last epilogue instead of after a build phase
                            emit_build(l + 1, b, hT_next, tbl_nx, edst_nx,
                                       tbl_owns[l + 1], tbl_fulls[l + 1])
                        elif kn["fusef"] and l == 5:
                            # final output rides layer 5's scatter stream
                            bi = b % OB
                            if bi == 0:
                                oo_t = sb.tile([128, OB, 256], f32, tag="oo")
                            emit_final(b, hT_next, oo_t, bi)
                if l < 5 and not kn["fuse"]:
                    for b in range(bpd):
                        emit_build(l + 1, b, hT_next, tbl_nx, edst_nx,
                                   tbl_owns[l + 1], tbl_fulls[l + 1])
                if l < 5:
                    tbl_sb, edst_all = tbl_nx, edst_nx
                hT = hT_next

            # ---- final: h6 = n'5 @ B5^-1 + b5; out = relu(h6) @ (W3_top+W3_bot)
            # batch 4 node blocks per output DMA: fewer HWDGE dispatches
            for b0 in ([] if kn["fusef"] else range(0, bpd, OB)):
                nb = min(OB, bpd - b0)
                oo = sb.tile([128, OB, 256], f32, tag="oo")
                for bi in range(nb):
                    b = b0 + bi
                    psf2 = psB.tile([128, 256], f32, tag="pst", bufs=1)
                    for dj in range(2):
                        for ki in range(2):
                            nc.tensor.matmul(
                                psf2[:, dj * 128 : (dj + 1) * 128],
                                lhsT=binv5sb[:, ki, dj],
                                rhs=hT[:, ki, b * 128 : (b + 1) * 128],
                                start=(ki == 0),
                                stop=(ki == 1),
                            )
                    zT = sb.tile([128, 2, 128], f32r, tag="zT")
                    for hh in range(2):
                        nc.vector.tensor_scalar(
                            out=zT[:, hh],
                            in0=psf2[:, hh * 128 : (hh + 1) * 128],
                            scalar1=b5sb[:, hh : hh + 1],
                            scalar2=0.0,
                            op0=OP.add,
                            op1=OP.max,
                        )
                    psf = psB.tile([128, 256], f32, tag="pshw", bufs=kn["pshwb"])
                    for ki in range(2):
                        nc.tensor.matmul(
                            psf[:],
                            lhsT=zT[:, ki],
                            rhs=w3ssb[:, ki, :],
                            start=(ki == 0),
                            stop=(ki == 1),
                        )
                    nc.vector.tensor_copy(oo[:, bi, :], psf[:])
                nc.sync.dma_start(
                    out[b0 * 128 : (b0 + nb) * 128, :].rearrange(
                        "(b p) c -> p b c", p=128
                    ),
                    oo[:, 0:nb, :],
                )

    nc.compile()
    return nc


# ---------------- entry point ----------------
DEFAULT_KNOBS = {}  # build()/prep() internal defaults apply


def kernel(**inputs):
    cfg = FULL
    in_maps, pos, cpb, nchunk = prep(inputs, cfg, knobs=DEFAULT_KNOBS)
    nc = build(cfg, cpb, nchunk, knobs=DEFAULT_KNOBS)
    from concourse import bass_utils

    res = bass_utils.run_bass_kernel_spmd(nc, in_maps, core_ids=list(range(NDEV)))
    outs = [res.results[dv]["out"] for dv in range(NDEV)]
    full = np.zeros((cfg.n, 256), np.float32)
    full[:] = np.stack(outs).reshape(NDEV * cfg.npd, 256)[pos]
    return full

